# revision 27
# baseline (speedup 1.0000x reference)
"""Trainium2 Bass kernel for a 2-layer mean-aggregation GraphSAGE GNN.

Strategy (8 NeuronCores, SPMD):
  - Nodes are assigned to (core, tile, slot) with degree balancing; each core
    owns 49 tiles x 128 slots = 6272 dst nodes and the ~100k edges into them.
  - Layer 1: per supertile, ONE dma_gather per src group pulls all x[src]
    rows; per dst tile ONE batched DVE op builds the binary segment matrix
    R[e, chunk, s] = (iota == dstslot) via broadcast APs; TensorE accumulates
    sum^T = sum_e M^T R in PSUM; the PSUM evacuation multiplies by the static
    per-slot 1/deg row (RD) to get mean^T.
    H^T = relu(W1_l @ mean^T + W1_r @ x^T + b1) via matmuls + fused ScalarE.
  - g = h @ W2_l^T computed per tile (node-major), copied out on ScalarE,
    written to DRAM and AllGather'd across cores (bf16, split lo/hi).
  - Layer 2: same gather/segment pipeline over g rows into psSeg; evacuate
    mean2 = psSeg * RD on DVE; psO = W2_r @ H^T; out = (psO + b2) + mean2.
Host does index-only preprocessing (permutation, edge chunking, 1/deg) and
the final unshard/transpose.
"""

import functools
import numpy as np

N_CORES = 8
TILES = 49  # tiles per core
TILE = 128
SHARD = TILES * TILE  # 6272
SUPER = 7  # tiles per supertile (gather-call granularity)
N_SUPER = TILES // SUPER  # 7
LO_SUPERS = 3  # supertiles in the "lo" AllGather split (early, hidden under L1)
LO_ROWS = LO_SUPERS * SUPER * TILE  # 3584
HI_ROWS = SHARD - LO_ROWS  # 2688
SPLIT16 = 32768  # int16 index limit for layer-1 x gather
GATHER_CHUNKS = 8  # 128-row chunks per dma_gather call (1024-idx SWDGE ring ceiling)


def _ceil_div(a, b):
    return -(-a // b)


def _wrap_idxs(idx_flat):
    """Wrap a flat int16 index list into the [128, n/16] dma_gather layout:
    index i lives at [i%16, i//16], replicated across the 8 groups of 16
    partitions."""
    n = len(idx_flat)
    assert n % 16 == 0
    w = np.asarray(idx_flat, np.int16).reshape(n // 16, 16).T  # [16, n/16]
    return np.tile(w, (8, 1))  # [128, n/16]


def _preprocess(x, edge_index, n_nodes):
    """Index-only host preprocessing: node permutation, per-core edge chunk
    streams for both layers, degree reciprocals.  Returns a dict of
    per-core/shared arrays plus layout metadata."""
    import ml_dtypes

    src = np.asarray(edge_index[0], np.int64)
    dst = np.asarray(edge_index[1], np.int64)
    E = src.shape[0]

    deg = np.bincount(dst, minlength=n_nodes).astype(np.int64)
    rdeg = (1.0 / np.maximum(deg, 1)).astype(np.float32)

    # Degree-balanced permutation: sort nodes by degree desc, deal round-robin
    # over the 392 global tiles; node -> (core, tile, slot).
    order = np.argsort(-deg, kind="stable")
    g_tile = np.empty(n_nodes, np.int64)   # global tile of node
    g_slot = np.empty(n_nodes, np.int64)   # slot within tile
    n_gtiles = N_CORES * TILES
    idx = np.arange(n_nodes)
    g_tile[order] = idx % n_gtiles
    g_slot[order] = idx // n_gtiles
    core_of = g_tile // TILES
    tile_of = g_tile % TILES
    row_of = tile_of * TILE + g_slot  # row within core shard [0, SHARD)

    e_core = core_of[dst]
    e_tile = tile_of[dst]
    e_slot = g_slot[dst]

    # Per-core static 1/deg row in slot order (pad slots -> 0), replicated
    # across partitions for the DVE evacuation multiply.
    rd = np.zeros((N_CORES, SHARD), np.float32)
    rd[core_of, row_of] = rdeg
    rd_full = np.broadcast_to(rd[:, None, :], (N_CORES, 128, SHARD)) \
        .astype(ml_dtypes.bfloat16)

    # Layer-1 groups: by src id vs int16 limit.
    l1_grp = (src >= SPLIT16).astype(np.int64)  # 0 = lo (idx=src), 1 = hi
    l1_idx = np.where(l1_grp == 0, src, src - SPLIT16)

    # Layer-2 groups: by gathered-g row (AllGather split layout).
    s_core = core_of[src]
    s_row = row_of[src]
    l2_grp = (s_row >= LO_ROWS).astype(np.int64)
    l2_idx = np.where(l2_grp == 0, s_core * LO_ROWS + s_row,
                      s_core * HI_ROWS + (s_row - LO_ROWS))

    def build_layer(grp, gidx):
        """Compute per-(core,tile,group) edge lists; fixed chunk budgets CA/CB
        (max over all cores/tiles); build the idx stream in supertile
        gather-call order and the dst-slot stream in tile-major chunk order
        (lo chunks then hi chunks per tile) for the batched R build."""
        counts = np.zeros((N_CORES, TILES, 2), np.int64)
        np.add.at(counts, (e_core, e_tile, grp), 1)
        CA = int(_ceil_div(counts[:, :, 0].max(), TILE))
        CB = int(_ceil_div(counts[:, :, 1].max(), TILE))
        # bucket edges
        key = (e_core * TILES + e_tile) * 2 + grp
        eorder = np.argsort(key * (2 * E) + gidx, kind="stable")  # sorted by key then src for DMA locality
        sorted_key = key[eorder]
        starts = np.searchsorted(sorted_key, np.arange(N_CORES * TILES * 2))
        ends = np.searchsorted(sorted_key, np.arange(N_CORES * TILES * 2) + 1)

        NCHUNK = TILES * (CA + CB)
        idx_cols_per_chunk = TILE // 16  # 8
        idx_arr = np.zeros((N_CORES, 128, NCHUNK * idx_cols_per_chunk), np.int16)
        ds_arr = np.full((N_CORES, 128, NCHUNK), -1.0, np.float32)

        for c in range(N_CORES):
            flat_idx = np.zeros(NCHUNK * TILE, np.int16)
            gc = 0  # global chunk cursor within core stream
            for S in range(N_SUPER):
                for g in range(2):
                    nch = CA if g == 0 else CB
                    for t0 in range(SUPER):
                        t = S * SUPER + t0
                        k = ((c * TILES + t) * 2) + g
                        es = eorder[starts[k]:ends[k]]
                        n_e = len(es)
                        assert n_e <= nch * TILE
                        span = slice(gc * TILE, gc * TILE + n_e)
                        flat_idx[span] = gidx[es].astype(np.int16)
                        pp = np.arange(n_e)
                        tb = t * (CA + CB) + (0 if g == 0 else CA)
                        ds_arr[c, pp % 128, tb + pp // 128] = e_slot[es]
                        gc += nch
            idx_arr[c] = _wrap_idxs(flat_idx)
        return dict(CA=CA, CB=CB, idx=idx_arr,
                    ds=ds_arr.astype(ml_dtypes.bfloat16))

    l1 = build_layer(l1_grp, l1_idx)
    l2 = build_layer(l2_grp, l2_idx)

    # Per-core x^T in slot order (zeros for pad slots).
    din = x.shape[1]
    xT = np.zeros((N_CORES, din, SHARD), np.float32)
    xT[core_of, :, row_of] = np.asarray(x, np.float32)  # fancy: for each node
    xT_bf = xT.astype(ml_dtypes.bfloat16)

    meta = dict(l1=l1, l2=l2, xT=xT_bf, rd=rd_full,
                core_of=core_of, row_of=row_of)
    return meta


@functools.lru_cache(maxsize=2)
def _build_program(din, dh, dout, CA1, CB1, CA2, CB2, n_lo, n_hi,
                   do_cc=True, do_c=True, shared_g=True):
    """Build the SPMD Bass/Tile program.  All shapes static."""
    import concourse.bacc as bacc
    import concourse.mybir as mybir
    import concourse.tile as tile
    from concourse.library_config import mlp

    bf16 = mybir.dt.bfloat16
    f32 = mybir.dt.float32
    i16 = mybir.dt.int16

    NC1 = TILES * (CA1 + CB1)
    NC2 = TILES * (CA2 + CB2)
    W1 = NC1 * 8  # idx cols (TILE/16 per chunk)
    W2 = NC2 * 8

    nc = bacc.Bacc("TRN2", target_bir_lowering=False, debug=False,
                   num_devices=N_CORES, num_swdge_queues=4)

    # ---- I/O tensors ----
    xg = nc.dram_tensor("xg", [n_lo + n_hi, din], bf16, kind="ExternalInput")
    xT_d = nc.dram_tensor("xT", [din, SHARD], bf16, kind="ExternalInput")
    idx1_d = nc.dram_tensor("idx1", [128, W1], i16, kind="ExternalInput")
    idx2_d = nc.dram_tensor("idx2", [128, W2], i16, kind="ExternalInput")
    ds1_d = nc.dram_tensor("ds1", [128, NC1], bf16, kind="ExternalInput")
    ds2_d = nc.dram_tensor("ds2", [128, NC2], bf16, kind="ExternalInput")
    rd_d = nc.dram_tensor("rd", [128, SHARD], bf16, kind="ExternalInput")
    w1lT_d = nc.dram_tensor("w1lT", [din, dh], bf16, kind="ExternalInput")
    w1rT_d = nc.dram_tensor("w1rT", [din, dh], bf16, kind="ExternalInput")
    w2lT_d = nc.dram_tensor("w2lT", [128, dh // 128, dout], bf16, kind="ExternalInput")
    w2rT_d = nc.dram_tensor("w2rT", [128, dh // 128, dout], bf16, kind="ExternalInput")
    b1_d = nc.dram_tensor("b1", [128, dh // 128], f32, kind="ExternalInput")
    b2_d = nc.dram_tensor("b2", [128, 1], f32, kind="ExternalInput")
    iota_d = nc.dram_tensor("iota", [128, 128], bf16, kind="ExternalInput")
    outT_d = nc.dram_tensor("outT", [dout, SHARD], f32, kind="ExternalOutput")

    # internal DRAM
    gl_lo = nc.dram_tensor("gl_lo", [LO_ROWS, dout], bf16)
    gl_hi = nc.dram_tensor("gl_hi", [HI_ROWS, dout], bf16)
    _aspace = "Shared" if shared_g else None
    gf_lo = nc.dram_tensor("gf_lo", [N_CORES * LO_ROWS, dout], bf16,
                           addr_space=_aspace)
    gf_hi = nc.dram_tensor("gf_hi", [N_CORES * HI_ROWS, dout], bf16,
                           addr_space=_aspace)

    NH = dh // 128  # h halves (2)
    NCH1 = CA1 + CB1  # chunks per tile, layer 1
    NCH2 = CA2 + CB2

    with tile.TileContext(nc) as tc:
        with (
            tc.tile_pool(name="per", bufs=1) as per,       # persistent SBUF
            tc.tile_pool(name="rt", bufs=2) as rpool,      # R tiles (per dst tile)
            tc.tile_pool(name="mt", bufs=3) as mpool,      # meanT / evict tiles
            tc.tile_pool(name="stg", bufs=3) as spool,     # staging for DRAM writes
            tc.tile_pool(name="ps_seg", bufs=2, space="PSUM") as ps_seg,
            tc.tile_pool(name="ps_h", bufs=2, space="PSUM") as ps_h,
            tc.tile_pool(name="ps_g", bufs=2, space="PSUM") as ps_g,
            tc.tile_pool(name="ps_o", bufs=2, space="PSUM") as ps_o,
            tc.tile_pool(name="gath", bufs=2) as gpool,    # gather buffers
        ):
            # ---- persistent loads ----
            xT = per.tile([din, SHARD], bf16)
            idx1 = per.tile([128, W1], i16)
            idx2 = per.tile([128, W2], i16)
            ds1 = per.tile([128, NC1], bf16)
            ds2 = per.tile([128, NC2], bf16)
            rd = per.tile([128, SHARD], bf16)
            w1lT = per.tile([din, dh], bf16)
            w1rT = per.tile([din, dh], bf16)
            w2lT = per.tile([128, NH, dout], bf16)
            w2rT = per.tile([128, NH, dout], bf16)
            b1 = per.tile([128, NH], f32)
            b2 = per.tile([128, 1], f32)
            iota = per.tile([128, 128], bf16)
            HT = per.tile([128, NH, SHARD], bf16)

            nc.gpsimd.load_library(mlp)
            for t_sb, t_dr in [(idx1, idx1_d), (ds1, ds1_d), (iota, iota_d),
                               (xT, xT_d), (w1lT, w1lT_d), (w1rT, w1rT_d),
                               (w2lT, w2lT_d), (w2rT, w2rT_d), (b1, b1_d),
                               (b2, b2_d), (rd, rd_d), (idx2, idx2_d),
                               (ds2, ds2_d)]:
                nc.sync.dma_start(t_sb[:], t_dr[:])

            xg_lo = xg[0:n_lo, :]
            xg_hi = xg[n_lo:n_lo + n_hi, :]

            def build_R(ds, t, nch, tag):
                """One DVE op: R[e, k, s] = (iota[e,s] == ds[e, t*nch+k])."""
                R = rpool.tile([128, nch, 128], bf16, tag=tag)
                iota_bc = iota[:].unsqueeze(1).broadcast_to([128, nch, 128])
                ds_bc = ds[:, t * nch:(t + 1) * nch].unsqueeze(2) \
                    .broadcast_to([128, nch, 128])
                nc.vector.scalar_tensor_tensor(
                    R[:], iota_bc, 1.0, ds_bc,
                    mybir.AluOpType.mult, mybir.AluOpType.is_equal)
                return R

            # ================= Stage A: layer 1 + H + g =================
            for S in range(N_SUPER):
                mA = gpool.tile([128, SUPER * CA1, din], bf16, tag="mA")
                mB = gpool.tile([128, SUPER * CB1, din], bf16, tag="mB")
                ca_cols = SUPER * CA1 * 8
                cb_cols = SUPER * CB1 * 8
                col0 = S * (ca_cols + cb_cols)
                for buf, nch, src_ap, c0 in [(mA, SUPER * CA1, xg_lo, col0),
                                             (mB, SUPER * CB1, xg_hi, col0 + ca_cols)]:
                    for q0 in range(0, nch, GATHER_CHUNKS):
                        n = min(GATHER_CHUNKS, nch - q0)
                        nc.gpsimd.dma_gather(
                            buf[:, q0:q0 + n, :], src_ap,
                            idx1[:, c0 + q0 * 8:c0 + (q0 + n) * 8],
                            n * TILE, n * TILE, din,
                            single_packet=False)
                for t0 in range(SUPER):
                    t = S * SUPER + t0
                    R = build_R(ds1, t, NCH1, "R1")
                    psS = ps_seg.tile([128, 128], f32, tag="psS")
                    for k in range(NCH1):
                        buf = mA[:, t0 * CA1 + k, :] if k < CA1 else \
                            mB[:, t0 * CB1 + (k - CA1), :]
                        nc.tensor.matmul(psS[:], lhsT=buf, rhs=R[:, k, :],
                                         start=(k == 0), stop=(k == NCH1 - 1))
                    meanT = mpool.tile([128, 128], bf16, tag="meanT")
                    # mean^T = sum^T * (1/deg per slot column)
                    nc.vector.scalar_tensor_tensor(
                        meanT[:], psS[:], 1.0, rd[:, t * TILE:(t + 1) * TILE],
                        mybir.AluOpType.mult, mybir.AluOpType.mult)
                    # H^T halves
                    for j in range(NH):
                        psH = ps_h.tile([128, 128], f32, tag="psH")
                        nc.tensor.matmul(psH[:], lhsT=w1lT[:, j * 128:(j + 1) * 128],
                                         rhs=meanT[:], start=True, stop=False)
                        nc.tensor.matmul(psH[:], lhsT=w1rT[:, j * 128:(j + 1) * 128],
                                         rhs=xT[:, t * TILE:(t + 1) * TILE],
                                         start=False, stop=True)
                        nc.scalar.activation(HT[:, j, t * TILE:(t + 1) * TILE], psH[:],
                                             mybir.ActivationFunctionType.Relu,
                                             bias=b1[:, j:j + 1])
                    # g tile (node-major)
                    psG = ps_g.tile([128, 128], f32, tag="psG")
                    for j in range(NH):
                        nc.tensor.matmul(psG[:], lhsT=HT[:, j, t * TILE:(t + 1) * TILE],
                                         rhs=w2lT[:, j, :], start=(j == 0),
                                         stop=(j == NH - 1))
                    gT = spool.tile([128, dout], bf16, tag="gT")
                    nc.scalar.activation(gT[:], psG[:],
                                         mybir.ActivationFunctionType.Copy)
                    row = t * TILE
                    if row < LO_ROWS:
                        dst = gl_lo[row:row + TILE, :]
                    else:
                        dst = gl_hi[row - LO_ROWS:row - LO_ROWS + TILE, :]
                    nc.sync.dma_start(dst, gT[:])
                # Fire each AllGather as soon as its inputs are written, so
                # the ~65us collective barrier overlaps remaining L1 work.
                if do_cc and S == LO_SUPERS - 1:
                    nc.gpsimd.collective_compute(
                        "AllGather", mybir.AluOpType.bypass,
                        replica_groups=[list(range(N_CORES))],
                        ins=[gl_lo.ap().opt()], outs=[gf_lo.ap().opt()])
                if do_cc and S == N_SUPER - 1:
                    nc.gpsimd.collective_compute(
                        "AllGather", mybir.AluOpType.bypass,
                        replica_groups=[list(range(N_CORES))],
                        ins=[gl_hi.ap().opt()], outs=[gf_hi.ap().opt()])

            # ================= Stage C: layer 2 =================
            # Software-pipelined gather issue: A2 (gf_lo) calls run one
            # supertile ahead so the Q7 chain hides the AG-hi completion.
            def _issue_c(S, which, bufs_by_S):
                ca_cols = SUPER * CA2 * 8
                cb_cols = SUPER * CB2 * 8
                col0 = S * (ca_cols + cb_cols)
                if which == "A":
                    mA = gpool.tile([128, SUPER * CA2, dout], bf16, tag="mA")
                    bufs_by_S.setdefault(S, {})["A"] = mA
                    nch, src_ap, c0, buf = SUPER * CA2, gf_lo[:], col0, mA
                else:
                    mB = gpool.tile([128, SUPER * CB2, dout], bf16, tag="mB")
                    bufs_by_S.setdefault(S, {})["B"] = mB
                    nch, src_ap, c0, buf = SUPER * CB2, gf_hi[:], col0 + ca_cols, mB
                for q0 in range(0, nch, GATHER_CHUNKS):
                    n = min(GATHER_CHUNKS, nch - q0)
                    nc.gpsimd.dma_gather(
                        buf[:, q0:q0 + n, :], src_ap,
                        idx2[:, c0 + q0 * 8:c0 + (q0 + n) * 8],
                        n * TILE, n * TILE, dout,
                        single_packet=False)

            _c_bufs = {}
            if do_c:
                _issue_c(0, "A", _c_bufs)
            for S in (range(N_SUPER) if do_c else []):
                if S + 1 < N_SUPER:
                    _issue_c(S + 1, "A", _c_bufs)
                _issue_c(S, "B", _c_bufs)
                mA = _c_bufs[S]["A"]
                mB = _c_bufs[S]["B"]
                for t0 in range(SUPER):
                    t = S * SUPER + t0
                    R = build_R(ds2, t, NCH2, "R2")
                    psSeg = ps_seg.tile([128, 128], f32, tag="psS")
                    for k in range(NCH2):
                        buf = mA[:, t0 * CA2 + k, :] if k < CA2 else \
                            mB[:, t0 * CB2 + (k - CA2), :]
                        nc.tensor.matmul(psSeg[:], lhsT=buf, rhs=R[:, k, :],
                                         start=(k == 0), stop=(k == NCH2 - 1))
                    m2 = mpool.tile([128, 128], f32, tag="m2")
                    nc.vector.scalar_tensor_tensor(
                        m2[:], psSeg[:], 1.0, rd[:, t * TILE:(t + 1) * TILE],
                        mybir.AluOpType.mult, mybir.AluOpType.mult)
                    psO = ps_o.tile([128, 128], f32, tag="psO")
                    for j in range(NH):
                        nc.tensor.matmul(psO[:], lhsT=w2rT[:, j, :],
                                         rhs=HT[:, j, t * TILE:(t + 1) * TILE],
                                         start=(j == 0), stop=(j == NH - 1))
                    oT = spool.tile([128, 128], f32, tag="oT")
                    # out^T = (psO + b2) + mean2
                    nc.vector.scalar_tensor_tensor(
                        oT[:], psO[:], b2[:, 0:1], m2[:],
                        mybir.AluOpType.add, mybir.AluOpType.add)
                    nc.sync.dma_start(
                        outT_d[:, t * TILE:(t + 1) * TILE], oT[:])

    # Align each gather's SWDGE queue with the DMASW sem lane Tile assigned
    # (sem lane L is locked to one queue; use queue = L % num_queues).
    import re as _re
    n_fix = 0
    for bb in nc.main_func.blocks:
        for ins in bb.instructions:
            if isinstance(ins, mybir.InstDMAGatherAnt):
                lane = None
                si = ins.sync_info
                if si is not None:
                    for upd in list(si.on_update):
                        m = _re.match(r"DMASW(\d+)", getattr(upd, "ant_name", None) or "")
                        if m:
                            lane = int(m.group(1))
                if lane is not None:
                    ins.queue_num = lane % 4
                    n_fix += 1
    nc.compile()
    return nc


def kernel(x, edge_index, W1_l, b1_l, W1_r, W2_l, b2_l, W2_r):
    import ml_dtypes
    from concourse.bass_utils import run_bass_kernel_spmd

    x = np.asarray(x, np.float32)
    n_nodes, din = x.shape
    dh = W1_l.shape[0]
    dout = W2_l.shape[0]

    meta = _preprocess(x, edge_index, n_nodes)
    l1, l2 = meta["l1"], meta["l2"]

    n_lo = SPLIT16
    n_hi = n_nodes - SPLIT16
    nc = _build_program(din, dh, dout, l1["CA"], l1["CB"], l2["CA"], l2["CB"],
                        n_lo, n_hi)

    bf = ml_dtypes.bfloat16
    xg = x.astype(bf)
    w1lT = np.ascontiguousarray(np.asarray(W1_l, np.float32).T).astype(bf)  # [din, dh]
    w1rT = np.ascontiguousarray(np.asarray(W1_r, np.float32).T).astype(bf)
    # [dh, dout] -> [128, dh//128, dout]
    w2lT = np.ascontiguousarray(np.asarray(W2_l, np.float32).T).reshape(
        dh // 128, 128, dout).transpose(1, 0, 2).astype(bf)
    w2rT = np.ascontiguousarray(np.asarray(W2_r, np.float32).T).reshape(
        dh // 128, 128, dout).transpose(1, 0, 2).astype(bf)
    b1 = np.ascontiguousarray(
        np.asarray(b1_l, np.float32).reshape(dh // 128, 128).T)  # [128, nh]
    b2 = np.asarray(b2_l, np.float32).reshape(128, 1)
    iota = np.tile(np.arange(128, dtype=np.float32), (128, 1)).astype(bf)

    in_maps = []
    for c in range(N_CORES):
        in_maps.append({
            "xg": xg, "xT": meta["xT"][c],
            "idx1": l1["idx"][c], "idx2": l2["idx"][c],
            "ds1": l1["ds"][c], "ds2": l2["ds"][c],
            "rd": meta["rd"][c],
            "w1lT": w1lT, "w1rT": w1rT, "w2lT": w2lT, "w2rT": w2rT,
            "b1": b1, "b2": b2, "iota": iota,
        })

    res = run_bass_kernel_spmd(nc, in_maps, list(range(N_CORES)))

    out = np.empty((n_nodes, dout), np.float32)
    core_of, row_of = meta["core_of"], meta["row_of"]
    outTs = np.stack([np.asarray(res.results[c]["outT"], np.float32)
                      for c in range(N_CORES)])  # [8, dout, SHARD]
    out[:, :] = outTs[core_of, :, row_of]
    return out
